# revision 7
# baseline (speedup 1.0000x reference)
"""Trainium2 Bass kernel for nn_Backflow (gnn_message_passing).

Pure data-parallel: batch B=128 sharded over 8 NeuronCores (16 samples each).
Params replicated. No collectives.

Math per sample (N=64 electrons, D=128, M=8 nuclei):
  electron: h = xs_i * xs_j (all ordered pairs) -> 3-layer MLP -> z3[i,j]
            bf_elec[i] = sum_j z3[i,j]*(rs_i - rs_j)
            (diagonal included: diff=0 contributes nothing; z3 symmetric)
            bf_elec = rs * rowsum(Z) - Z @ rs
  nuclear:  g = MLP(xs) -> [N, M]; bf_nuc = rs*sum_m(g) - g @ coords
  cutoff:   prod_m f(|rs - coords_m|)
  out = rs + 1e-4 * cutoff * (bf_elec + bf_nuc)

softplus(x)-ln2 is handled by folding -ln2*W.sum(0) into the next layer bias.
"""

import sys

sys.path.insert(0, "/opt/trn_rl_repo")

import numpy as np

import concourse.bass as bass
import concourse.tile as tile
from concourse import bacc, mybir

LN2 = 0.6931471805599453
N_CORES = 8
B, N, D, M = 128, 64, 128, 8
BS = B // N_CORES          # samples per core
R = BS * N                 # rows per core = 1024
NCH = R // 128             # 128-row chunks per core = 8
F32 = mybir.dt.float32
AF = mybir.ActivationFunctionType
ALU = mybir.AluOpType

_CACHE = {}


def build_graph():
    nc = bacc.Bacc(
        "TRN2", target_bir_lowering=False, debug=False, num_devices=N_CORES
    )

    # ---- DRAM parameters (per-core shard views) ----
    def inp(name, shape):
        return nc.dram_tensor(name, shape, F32, kind="ExternalInput").ap()

    rs_d = inp("rs", [R, 3])
    xs_d = inp("xs", [R, D])
    we1_d = inp("We1", [128, 25])
    be1_d = inp("be1", [25, 1])
    we2_d = inp("We2", [25, 5])
    be2_d = inp("be2a", [5, 1])
    we3_d = inp("We3", [5, 1])
    be3_d = inp("be3a", [1, 1])
    wn1_d = inp("Wn1", [128, 51])
    bn1_d = inp("bn1", [51, 1])
    wn2_d = inp("Wn2", [51, 20])
    bn2_d = inp("bn2a", [20, 1])
    wn3_d = inp("Wn3", [20, 8])
    bn3_d = inp("bn3a", [8, 1])
    oc_d = inp("OC", [8, 4])            # col0: ones, cols1-3: coords
    cb_d = inp("coordsB", [128, 24])    # coords flattened, tiled over partitions
    eye_d = inp("eye", [128, 128])
    out_d = nc.dram_tensor("out", [R, 3], F32, kind="ExternalOutput").ap()

    with tile.TileContext(nc) as tc:
        _kernel_body(
            tc, rs_d, xs_d, we1_d, be1_d, we2_d, be2_d, we3_d, be3_d,
            wn1_d, bn1_d, wn2_d, bn2_d, wn3_d, bn3_d, oc_d, cb_d, eye_d, out_d,
        )
    nc.compile()
    return nc


def _kernel_body(tc, rs_d, xs_d, we1_d, be1_d, we2_d, be2_d, we3_d, be3_d,
                 wn1_d, bn1_d, wn2_d, bn2_d, wn3_d, bn3_d, oc_d, cb_d, eye_d,
                 out_d):
    nc = tc.nc
    from contextlib import ExitStack

    ctx = ExitStack()
    with ctx:
        consts = ctx.enter_context(tc.tile_pool(name="consts", bufs=1))
        datap = ctx.enter_context(tc.tile_pool(name="data", bufs=1))
        hpool = ctx.enter_context(tc.tile_pool(name="hp", bufs=2))
        z1pool = ctx.enter_context(tc.tile_pool(name="z1p", bufs=2))
        z2pool = ctx.enter_context(tc.tile_pool(name="z2p", bufs=2))
        z3pool = ctx.enter_context(tc.tile_pool(name="z3p", bufs=2))
        zpool = ctx.enter_context(tc.tile_pool(name="zp", bufs=2))
        smallp = ctx.enter_context(tc.tile_pool(name="smallp", bufs=2))
        psum = ctx.enter_context(
            tc.tile_pool(name="psum", bufs=6, space="PSUM")
        )

        def pst(p0, p1):
            return psum.tile([p0, p1], F32, tag="ps", name="ps")

        # ---- load constants ----
        def ctile(shape, src):
            t = consts.tile(shape, F32, tag=f"c{len(consts_list)}",
                            name=f"c{len(consts_list)}")
            nc.sync.dma_start(t[:], src)
            consts_list.append(t)
            return t

        consts_list = []
        we1 = ctile([128, 25], we1_d[:])
        be1 = ctile([25, 1], be1_d[:])
        we2 = ctile([25, 5], we2_d[:])
        be2 = ctile([5, 1], be2_d[:])
        we3 = ctile([5, 1], we3_d[:])
        be3 = ctile([1, 1], be3_d[:])
        wn1 = ctile([128, 51], wn1_d[:])
        bn1 = ctile([51, 1], bn1_d[:])
        wn2 = ctile([51, 20], wn2_d[:])
        bn2 = ctile([20, 1], bn2_d[:])
        wn3 = ctile([20, 8], wn3_d[:])
        bn3 = ctile([8, 1], bn3_d[:])
        oc = ctile([8, 4], oc_d[:])
        coordsB = ctile([128, 24], cb_d[:])
        eye = ctile([128, 128], eye_d[:])

        # rs rows: [128 part, chunk, 3]
        rs_sb = consts.tile([128, NCH, 3], F32, tag="rs")
        for c in range(NCH):
            nc.sync.dma_start(rs_sb[:, c, :], rs_d[128 * c:128 * (c + 1), :])

        # xs rows -> transpose to xsT [128(d), 1024(row)]
        xs_rows = datap.tile([128, NCH, 128], F32, tag="xsr")
        for c in range(NCH):
            nc.sync.dma_start(xs_rows[:, c, :], xs_d[128 * c:128 * (c + 1), :])
        xsT = datap.tile([128, R], F32, tag="xsT")
        for c in range(NCH):
            pT = pst(128, 128)
            nc.tensor.transpose(pT[:, 0:128], xs_rows[:, c, :], eye[:])
            nc.vector.tensor_copy(xsT[:, 128 * c:128 * (c + 1)], pT[:, 0:128])

        # ---- nuclear MLP over all rows ----
        g1 = datap.tile([51, R], F32, tag="g1")
        g2 = datap.tile([20, R], F32, tag="g2")
        g3 = datap.tile([8, R], F32, tag="g3")
        sc = datap.tile([4, R], F32, tag="sc")
        for n in range(R // 512):
            cols = slice(512 * n, 512 * (n + 1))
            p1 = pst(51, 512)
            nc.tensor.matmul(p1[:, 0:512], wn1[:], xsT[:, cols])
            # softplus(x+b) = ln(1 + exp(x+b)); -ln2 folded into next bias
            nc.scalar.activation(g1[:, cols], p1[:, 0:512], AF.Exp,
                                 bias=bn1[:, 0:1])
            nc.scalar.activation(g1[:, cols], g1[:, cols], AF.Ln, bias=1.0)
            p2 = pst(20, 512)
            nc.tensor.matmul(p2[:, 0:512], wn2[:], g1[:, cols])
            nc.scalar.activation(g2[:, cols], p2[:, 0:512], AF.Exp,
                                 bias=bn2[:, 0:1])
            nc.scalar.activation(g2[:, cols], g2[:, cols], AF.Ln, bias=1.0)
            p3 = pst(8, 512)
            nc.tensor.matmul(p3[:, 0:512], wn3[:], g2[:, cols])
            nc.scalar.activation(g3[:, cols], p3[:, 0:512], AF.Identity,
                                 bias=bn3[:, 0:1])
            p4 = pst(4, 512)
            nc.tensor.matmul(p4[:, 0:512], oc[:], g3[:, cols])
            nc.vector.tensor_copy(sc[:, cols], p4[:, 0:512])

        # bf accumulator [128, chunk, 3], cutoff d2 [128, 8*8]
        bf = datap.tile([128, NCH, 3], F32, tag="bf")
        d2 = datap.tile([128, NCH * M], F32, tag="d2")
        for c in range(NCH):
            # transpose sc chunk [4, 128] -> [128, 4]
            pT4 = pst(128, 4)
            nc.tensor.transpose(pT4[:, 0:4], sc[:, 128 * c:128 * (c + 1)],
                                eye[0:4, 0:4])
            sc4 = smallp.tile([128, 4], F32, tag="sc4")
            nc.vector.tensor_copy(sc4[:], pT4[:, 0:4])
            # bf_nuc = rs * sum_m g  -  g @ coords
            nc.vector.tensor_scalar(bf[:, c, :], rs_sb[:, c, :],
                                    sc4[:, 0:1], None, ALU.mult)
            nc.vector.tensor_sub(bf[:, c, :], bf[:, c, :], sc4[:, 1:4])
            # cutoff distances: diffs [128, m, 3]
            df = smallp.tile([128, M, 3], F32, tag="df")
            rs_b = rs_sb[:, c, :].unsqueeze(1).broadcast_to([128, M, 3])
            nc.vector.tensor_sub(
                df[:], rs_b, coordsB[:].rearrange("p (m c) -> p m c", c=3))
            nc.vector.tensor_mul(df[:], df[:], df[:])
            nc.vector.tensor_reduce(d2[:, M * c:M * (c + 1)], df[:],
                                    mybir.AxisListType.X, ALU.add)

        # cutoff = where(r/L < L, (r/L)^2(6-8(r/L)+3(r/L)^2), 1); L=0.5
        # r1 = 2*sqrt(d2) = sqrt(4*d2);  r1 < 0.5 <=> d2 < 1/64
        r1 = datap.tile([128, NCH * M], F32, tag="r1")
        nc.scalar.activation(r1[:], d2[:], AF.Sqrt, scale=4.0)
        pa = datap.tile([128, NCH * M], F32, tag="pa")
        nc.vector.tensor_scalar(pa[:], r1[:], 3.0, -8.0, ALU.mult, ALU.add)
        nc.vector.tensor_mul(pa[:], pa[:], r1[:])
        nc.vector.tensor_scalar(pa[:], pa[:], 6.0, None, ALU.add)
        nc.vector.tensor_mul(r1[:], r1[:], r1[:])
        nc.vector.tensor_mul(pa[:], pa[:], r1[:])
        msk = datap.tile([128, NCH * M], mybir.dt.uint8, tag="msk")
        nc.vector.tensor_scalar(msk[:], d2[:], 1.0 / 64.0, None, ALU.is_lt)
        cu = datap.tile([128, NCH * M], F32, tag="cu")
        nc.vector.memset(cu[:], 1.0)
        nc.vector.copy_predicated(cu[:], msk[:], pa[:])
        # product over m (pairwise tree), cu viewed [128, c, m]
        cuv = cu[:].rearrange("p (c m) -> p c m", m=M)
        t1 = datap.tile([128, NCH, 4], F32, tag="t1")
        nc.vector.tensor_mul(t1[:], cuv[:, :, 0:4], cuv[:, :, 4:8])
        t2 = datap.tile([128, NCH, 2], F32, tag="t2")
        nc.vector.tensor_mul(t2[:], t1[:, :, 0:2], t1[:, :, 2:4])
        cut = datap.tile([128, NCH], F32, tag="cut")
        nc.vector.tensor_mul(
            cut[:].unsqueeze(2), t2[:, :, 0:1], t2[:, :, 1:2])

        # ---- electron pair MLP, per sample ----
        for c in range(NCH):
            zsb = zpool.tile([128, 64], F32, tag="zsb")
            for h in range(2):
                s = 2 * c + h
                scol = slice(64 * s, 64 * (s + 1))
                # H = xs_i * xs_j  for all pairs
                hT = hpool.tile([128, 64, 64], F32, tag="H")
                xi = xsT[:, scol].unsqueeze(2).broadcast_to([128, 64, 64])
                xj = xsT[:, scol].unsqueeze(1).broadcast_to([128, 64, 64])
                nc.vector.tensor_mul(hT[:], xi, xj)
                hflat = hT[:].rearrange("p i j -> p (i j)")
                z1 = z1pool.tile([25, 4096], F32, tag="z1")
                z2 = z2pool.tile([5, 4096], F32, tag="z2")
                z3 = z3pool.tile([1, 4096], F32, tag="z3")
                for n in range(8):
                    cols = slice(512 * n, 512 * (n + 1))
                    p1 = pst(25, 512)
                    nc.tensor.matmul(p1[:, 0:512], we1[:], hflat[:, cols])
                    nc.scalar.activation(z1[:, cols], p1[:, 0:512],
                                         AF.Exp, bias=be1[:, 0:1])
                    nc.scalar.activation(z1[:, cols], z1[:, cols],
                                         AF.Ln, bias=1.0)
                    p2 = pst(5, 512)
                    nc.tensor.matmul(p2[:, 0:512], we2[:], z1[:, cols])
                    nc.scalar.activation(z2[:, cols], p2[:, 0:512],
                                         AF.Exp, bias=be2[:, 0:1])
                    nc.scalar.activation(z2[:, cols], z2[:, cols],
                                         AF.Ln, bias=1.0)
                    p3 = pst(1, 512)
                    nc.tensor.matmul(p3[:, 0:512], we3[:], z2[:, cols])
                    nc.scalar.activation(z3[:, cols], p3[:, 0:512],
                                         AF.Identity, bias=be3[:, 0:1])
                # scatter z3 [1, 4096] -> Z [64, 64] rows
                nc.sync.dma_start(zsb[64 * h:64 * (h + 1), :], z3[:])
            # rowsum and Z @ rs for both samples in chunk
            s2 = smallp.tile([128, 1], F32, tag="s2")
            nc.vector.tensor_reduce(s2[:], zsb[:], mybir.AxisListType.X,
                                    ALU.add)
            pE = pst(128, 3)
            for h in range(2):
                pr = slice(64 * h, 64 * (h + 1))
                nc.tensor.matmul(pE[pr, 0:3], zsb[pr, :], rs_sb[pr, c, :],
                                 tile_position=(64 * h, 64 * h))
            # bf += rs*rowsum - Z@rs
            tmp = smallp.tile([128, 3], F32, tag="tmpE")
            nc.vector.tensor_scalar(tmp[:], rs_sb[:, c, :], s2[:, 0:1],
                                    None, ALU.mult)
            nc.vector.tensor_sub(tmp[:], tmp[:], pE[:, 0:3])
            nc.vector.tensor_add(bf[:, c, :], bf[:, c, :], tmp[:])

        # ---- final combine + store ----
        for c in range(NCH):
            o = smallp.tile([128, 3], F32, tag="oc")
            nc.vector.tensor_scalar(o[:], bf[:, c, :], cut[:, c:c + 1],
                                    1e-4, ALU.mult, ALU.mult)
            nc.vector.tensor_add(o[:], o[:], rs_sb[:, c, :])
            nc.sync.dma_start(out_d[128 * c:128 * (c + 1), :], o[:])


def prep_inputs(rs, xs, coords, We1, be1, We2, be2, We3, be3,
                Wn1, bn1, Wn2, bn2, Wn3, bn3):
    """Host-side: shard rs/xs over cores, fold -ln2 into biases, pack."""
    f = np.float32
    rs = np.asarray(rs, f)
    xs = np.asarray(xs, f)
    coords = np.asarray(coords, f)
    be2a = (np.asarray(be2, f) - LN2 * np.asarray(We2, f).sum(0)).reshape(5, 1)
    be3a = (np.asarray(be3, f) - LN2 * np.asarray(We3, f).sum(0)).reshape(1, 1)
    bn2a = (np.asarray(bn2, f) - LN2 * np.asarray(Wn2, f).sum(0)).reshape(20, 1)
    bn3a = (np.asarray(bn3, f) - LN2 * np.asarray(Wn3, f).sum(0)).reshape(8, 1)
    oc = np.concatenate([np.ones((8, 1), f), coords], axis=1)
    coordsB = np.tile(coords.reshape(1, 24), (128, 1)).astype(f)
    eye = np.eye(128, dtype=f)
    shared = dict(
        We1=np.ascontiguousarray(We1, f),
        be1=np.asarray(be1, f).reshape(25, 1),
        We2=np.ascontiguousarray(We2, f), be2a=be2a,
        We3=np.ascontiguousarray(We3, f), be3a=be3a,
        Wn1=np.ascontiguousarray(Wn1, f),
        bn1=np.asarray(bn1, f).reshape(51, 1),
        Wn2=np.ascontiguousarray(Wn2, f), bn2a=bn2a,
        Wn3=np.ascontiguousarray(Wn3, f), bn3a=bn3a,
        OC=np.ascontiguousarray(oc), coordsB=coordsB, eye=eye,
    )
    in_maps = []
    for i in range(N_CORES):
        m = dict(shared)
        m["rs"] = np.ascontiguousarray(rs[BS * i:BS * (i + 1)].reshape(R, 3))
        m["xs"] = np.ascontiguousarray(xs[BS * i:BS * (i + 1)].reshape(R, D))
        in_maps.append(m)
    return in_maps


def get_graph():
    if "nc" not in _CACHE:
        _CACHE["nc"] = build_graph()
    return _CACHE["nc"]


def kernel(**inputs):
    from concourse.bass_utils import run_bass_kernel_spmd

    nc = get_graph()
    in_maps = prep_inputs(**inputs)
    res = run_bass_kernel_spmd(nc, in_maps, core_ids=list(range(N_CORES)))
    outs = [res.results[i]["out"].reshape(BS, N, 3) for i in range(N_CORES)]
    return np.concatenate(outs, axis=0)


# revision 16
# speedup vs baseline: 15.2405x; 15.2405x over previous
"""Trainium2 Bass kernel for nn_Backflow (gnn_message_passing).

Pure data-parallel: batch B=128 sharded over 8 NeuronCores (16 samples each).
Params replicated. No collectives.

Math per sample (N=64 electrons, D=128, M=8 nuclei):
  electron: h = xs_i * xs_j (all ordered pairs) -> 3-layer MLP -> z3[i,j]
            bf_elec[i] = sum_j z3[i,j]*(rs_i - rs_j)
            (diagonal included: diff=0 contributes nothing; z3 symmetric)
            bf_elec = rs * rowsum(Z) - Z @ rs
  nuclear:  g = MLP(xs) -> [N, M]; bf_nuc = rs*sum_m(g) - g @ coords
  cutoff:   prod_m f(|rs - coords_m|)
  out = rs + 1e-4 * cutoff * (bf_elec + bf_nuc)

softplus(x)-ln2 is handled by folding -ln2*W.sum(0) into the next layer bias.
"""

import sys

sys.path.insert(0, "/opt/trn_rl_repo")

import numpy as np

import concourse.bass as bass
import concourse.tile as tile
from concourse import bacc, mybir

LN2 = 0.6931471805599453
N_CORES = 8
B, N, D, M = 128, 64, 128, 8
BS = B // N_CORES          # samples per core
R = BS * N                 # rows per core = 1024
NCH = R // 128             # 128-row chunks per core = 8
F32 = mybir.dt.float32
BF16 = mybir.dt.bfloat16
AF = mybir.ActivationFunctionType
ALU = mybir.AluOpType

_CACHE = {}


def _patch_act_tables():
    """Force exp/ln/identity into one act-func set so bacc doesn't
    reload the LUT before (almost) every ACTIVATE (measured 521 loads,
    668us). Keep only two sets selectable; order (= set ids) preserved."""
    import concourse.bacc as bacc_mod
    from concourse import hw_specs

    if getattr(bacc_mod.get_activation_tables, "_patched", False):
        return
    orig = hw_specs.get_activation_tables
    keep = {"natural_log_exp_and_others", "sqrt_and_others"}

    def patched(arch):
        return {k: (v if k in keep else set()) for k, v in orig(arch).items()}

    patched._patched = True
    bacc_mod.get_activation_tables = patched


def build_graph():
    _patch_act_tables()
    nc = bacc.Bacc(
        "TRN2", target_bir_lowering=False, debug=False, num_devices=N_CORES
    )

    # ---- DRAM parameters (per-core shard views) ----
    def inp(name, shape, dt=F32):
        return nc.dram_tensor(name, shape, dt, kind="ExternalInput").ap()

    rs_d = inp("rs", [R, 3])
    xs_d = inp("xs", [R, D])
    we1_d = inp("We1", [128, 25], BF16)
    be1_d = inp("be1", [25, 1])
    we2_d = inp("We2", [25, 5], BF16)
    be2_d = inp("be2a", [5, 1])
    we3_d = inp("We3", [5, 1], BF16)
    be3_d = inp("be3a", [1, 1])
    wn1_d = inp("Wn1", [128, 51], BF16)
    bn1_d = inp("bn1", [51, 1])
    wn2_d = inp("Wn2", [51, 20], BF16)
    bn2_d = inp("bn2a", [20, 1])
    wn3_d = inp("Wn3", [20, 8], BF16)
    bn3_d = inp("bn3a", [8, 1])
    oc_d = inp("OC", [8, 4], BF16)      # col0: ones, cols1-3: coords
    cb_d = inp("coordsB", [128, 24])    # coords flattened, tiled over partitions
    eye_d = inp("eye", [128, 128], BF16)
    ey4_d = inp("eye4", [4, 4])
    out_d = nc.dram_tensor("out", [R, 3], F32, kind="ExternalOutput").ap()

    with tile.TileContext(nc) as tc:
        _kernel_body(
            tc, rs_d, xs_d, we1_d, be1_d, we2_d, be2_d, we3_d, be3_d,
            wn1_d, bn1_d, wn2_d, bn2_d, wn3_d, bn3_d, oc_d, cb_d, eye_d,
            ey4_d, out_d,
        )
    nc.compile()
    return nc


def _kernel_body(tc, rs_d, xs_d, we1_d, be1_d, we2_d, be2_d, we3_d, be3_d,
                 wn1_d, bn1_d, wn2_d, bn2_d, wn3_d, bn3_d, oc_d, cb_d, eye_d,
                 ey4_d, out_d):
    nc = tc.nc
    from contextlib import ExitStack

    ctx = ExitStack()
    with ctx:
        consts = ctx.enter_context(tc.tile_pool(name="consts", bufs=1))
        datap = ctx.enter_context(tc.tile_pool(name="data", bufs=1))
        hpool = ctx.enter_context(tc.tile_pool(name="hp", bufs=2))
        z1pool = ctx.enter_context(tc.tile_pool(name="z1p", bufs=2))
        z2pool = ctx.enter_context(tc.tile_pool(name="z2p", bufs=2))
        z3pool = ctx.enter_context(tc.tile_pool(name="z3p", bufs=2))
        zpool = ctx.enter_context(tc.tile_pool(name="zp", bufs=2))
        smallp = ctx.enter_context(tc.tile_pool(name="smallp", bufs=2))
        psum = ctx.enter_context(
            tc.tile_pool(name="psum", bufs=6, space="PSUM")
        )

        def pst(p0, p1, dt=F32):
            return psum.tile([p0, p1], dt, tag="ps", name="ps")

        # ---- load constants ----
        def ctile(shape, src, dt=F32):
            t = consts.tile(shape, dt, tag=f"c{len(consts_list)}",
                            name=f"c{len(consts_list)}")
            nc.sync.dma_start(t[:], src)
            consts_list.append(t)
            return t

        consts_list = []
        we1 = ctile([128, 25], we1_d[:], BF16)
        be1 = ctile([25, 1], be1_d[:])
        we2 = ctile([25, 5], we2_d[:], BF16)
        be2 = ctile([5, 1], be2_d[:])
        we3 = ctile([5, 1], we3_d[:], BF16)
        be3 = ctile([1, 1], be3_d[:])
        wn1 = ctile([128, 51], wn1_d[:], BF16)
        bn1 = ctile([51, 1], bn1_d[:])
        wn2 = ctile([51, 20], wn2_d[:], BF16)
        bn2 = ctile([20, 1], bn2_d[:])
        wn3 = ctile([20, 8], wn3_d[:], BF16)
        bn3 = ctile([8, 1], bn3_d[:])
        oc = ctile([8, 4], oc_d[:], BF16)
        coordsB = ctile([128, 24], cb_d[:])
        eye = ctile([128, 128], eye_d[:], BF16)
        eye4 = ctile([4, 4], ey4_d[:])

        # rs rows: [128 part, chunk, 3]
        rs_sb = consts.tile([128, NCH, 3], F32, tag="rs")
        for c in range(NCH):
            nc.sync.dma_start(rs_sb[:, c, :], rs_d[128 * c:128 * (c + 1), :])

        # xs rows -> bf16 -> transpose to xsT [128(d), 1024(row)] bf16
        xs_rows = datap.tile([128, NCH, 128], F32, tag="xsr")
        for c in range(NCH):
            nc.sync.dma_start(xs_rows[:, c, :], xs_d[128 * c:128 * (c + 1), :])
        xs_bf = datap.tile([128, NCH, 128], BF16, tag="xsb")
        nc.vector.tensor_copy(xs_bf[:], xs_rows[:])
        xsT = datap.tile([128, R], BF16, tag="xsT")
        for c in range(NCH):
            pT = pst(128, 128, BF16)
            nc.tensor.transpose(pT[:, 0:128], xs_bf[:, c, :], eye[:])
            nc.vector.tensor_copy(xsT[:, 128 * c:128 * (c + 1)], pT[:, 0:128])
        # bf16 copy of rs rows for the Z@rs matmuls
        rs_bf = consts.tile([128, NCH, 3], BF16, tag="rsbf")
        nc.vector.tensor_copy(rs_bf[:], rs_sb[:])

        # ---- nuclear MLP over all rows ----
        g1 = datap.tile([51, R], BF16, tag="g1")
        g2 = datap.tile([20, R], BF16, tag="g2")
        g3 = datap.tile([8, R], BF16, tag="g3")
        sc = datap.tile([4, R], F32, tag="sc")
        for n in range(R // 512):
            cols = slice(512 * n, 512 * (n + 1))
            p1 = pst(51, 512)
            nc.tensor.matmul(p1[:, 0:512], wn1[:], xsT[:, cols])
            # softplus(x+b) = ln(1 + exp(x+b)); -ln2 folded into next bias
            nc.scalar.activation(g1[:, cols], p1[:, 0:512], AF.Exp,
                                 bias=bn1[:, 0:1])
            nc.scalar.activation(g1[:, cols], g1[:, cols], AF.Ln, bias=1.0)
            p2 = pst(20, 512)
            nc.tensor.matmul(p2[:, 0:512], wn2[:], g1[:, cols])
            nc.scalar.activation(g2[:, cols], p2[:, 0:512], AF.Exp,
                                 bias=bn2[:, 0:1])
            nc.scalar.activation(g2[:, cols], g2[:, cols], AF.Ln, bias=1.0)
            p3 = pst(8, 512)
            nc.tensor.matmul(p3[:, 0:512], wn3[:], g2[:, cols])
            nc.scalar.activation(g3[:, cols], p3[:, 0:512], AF.Identity,
                                 bias=bn3[:, 0:1])
            p4 = pst(4, 512)
            nc.tensor.matmul(p4[:, 0:512], oc[:], g3[:, cols])
            nc.vector.tensor_copy(sc[:, cols], p4[:, 0:512])

        # bf accumulator [128, chunk, 3], cutoff d2 [128, 8*8]
        bf = datap.tile([128, NCH, 3], F32, tag="bf")
        d2 = datap.tile([128, NCH * M], F32, tag="d2")
        for c in range(NCH):
            # transpose sc chunk [4, 128] -> [128, 4]
            pT4 = pst(128, 4)
            nc.tensor.transpose(pT4[:, 0:4], sc[:, 128 * c:128 * (c + 1)],
                                eye4[:])
            sc4 = smallp.tile([128, 4], F32, tag="sc4")
            nc.vector.tensor_copy(sc4[:], pT4[:, 0:4])
            # bf_nuc = rs * sum_m g  -  g @ coords
            nc.vector.tensor_scalar(bf[:, c, :], rs_sb[:, c, :],
                                    sc4[:, 0:1], None, ALU.mult)
            nc.vector.tensor_sub(bf[:, c, :], bf[:, c, :], sc4[:, 1:4])
            # cutoff distances: diffs [128, m, 3]
            df = smallp.tile([128, M, 3], F32, tag="df")
            rs_b = rs_sb[:, c, :].unsqueeze(1).broadcast_to([128, M, 3])
            nc.vector.tensor_sub(
                df[:], rs_b, coordsB[:].rearrange("p (m c) -> p m c", c=3))
            nc.vector.tensor_mul(df[:], df[:], df[:])
            nc.vector.tensor_reduce(d2[:, M * c:M * (c + 1)], df[:],
                                    mybir.AxisListType.X, ALU.add)

        # cutoff = where(r/L < L, (r/L)^2(6-8(r/L)+3(r/L)^2), 1); L=0.5
        # r1 = 2*sqrt(d2) = sqrt(4*d2);  r1 < 0.5 <=> d2 < 1/64
        r1 = datap.tile([128, NCH * M], F32, tag="r1")
        nc.scalar.activation(r1[:], d2[:], AF.Sqrt, scale=4.0)
        pa = datap.tile([128, NCH * M], F32, tag="pa")
        nc.vector.tensor_scalar(pa[:], r1[:], 3.0, -8.0, ALU.mult, ALU.add)
        nc.vector.tensor_mul(pa[:], pa[:], r1[:])
        nc.vector.tensor_scalar(pa[:], pa[:], 6.0, None, ALU.add)
        nc.vector.tensor_mul(r1[:], r1[:], r1[:])
        nc.vector.tensor_mul(pa[:], pa[:], r1[:])
        msk = datap.tile([128, NCH * M], mybir.dt.uint8, tag="msk")
        nc.vector.tensor_scalar(msk[:], d2[:], 1.0 / 64.0, None, ALU.is_lt)
        cu = datap.tile([128, NCH * M], F32, tag="cu")
        nc.vector.memset(cu[:], 1.0)
        nc.vector.copy_predicated(cu[:], msk[:], pa[:])
        # product over m (pairwise tree), cu viewed [128, c, m]
        cuv = cu[:].rearrange("p (c m) -> p c m", m=M)
        t1 = datap.tile([128, NCH, 4], F32, tag="t1")
        nc.vector.tensor_mul(t1[:], cuv[:, :, 0:4], cuv[:, :, 4:8])
        t2 = datap.tile([128, NCH, 2], F32, tag="t2")
        nc.vector.tensor_mul(t2[:], t1[:, :, 0:2], t1[:, :, 2:4])
        cut = datap.tile([128, NCH], F32, tag="cut")
        nc.vector.tensor_mul(
            cut[:].unsqueeze(2), t2[:, :, 0:1], t2[:, :, 1:2])

        # ---- electron pair MLP, per sample ----
        for c in range(NCH):
            zsb = zpool.tile([128, 64], BF16, tag="zsb")
            for h in range(2):
                s = 2 * c + h
                scol = slice(64 * s, 64 * (s + 1))
                # H = xs_i * xs_j  for all pairs
                hT = hpool.tile([128, 64, 64], BF16, tag="H")
                xi = xsT[:, scol].unsqueeze(2).broadcast_to([128, 64, 64])
                xj = xsT[:, scol].unsqueeze(1).broadcast_to([128, 64, 64])
                nc.vector.tensor_mul(hT[:], xi, xj)
                hflat = hT[:].rearrange("p i j -> p (i j)")
                z1 = z1pool.tile([25, 4096], BF16, tag="z1")
                z2 = z2pool.tile([5, 4096], BF16, tag="z2")
                z3 = z3pool.tile([1, 4096], BF16, tag="z3")
                for n in range(8):
                    cols = slice(512 * n, 512 * (n + 1))
                    p1 = pst(25, 512)
                    nc.tensor.matmul(p1[:, 0:512], we1[:], hflat[:, cols])
                    nc.scalar.activation(z1[:, cols], p1[:, 0:512],
                                         AF.Exp, bias=be1[:, 0:1])
                    nc.scalar.activation(z1[:, cols], z1[:, cols],
                                         AF.Ln, bias=1.0)
                    p2 = pst(5, 512)
                    nc.tensor.matmul(p2[:, 0:512], we2[:], z1[:, cols])
                    nc.scalar.activation(z2[:, cols], p2[:, 0:512],
                                         AF.Exp, bias=be2[:, 0:1])
                    nc.scalar.activation(z2[:, cols], z2[:, cols],
                                         AF.Ln, bias=1.0)
                    p3 = pst(1, 512)
                    nc.tensor.matmul(p3[:, 0:512], we3[:], z2[:, cols])
                    nc.scalar.activation(z3[:, cols], p3[:, 0:512],
                                         AF.Identity, bias=be3[:, 0:1])
                # scatter z3 [1, 4096] -> Z [64, 64] rows
                nc.sync.dma_start(zsb[64 * h:64 * (h + 1), :], z3[:])
            # rowsum and Z @ rs for both samples in chunk
            s2 = smallp.tile([128, 1], F32, tag="s2")
            nc.vector.tensor_reduce(s2[:], zsb[:], mybir.AxisListType.X,
                                    ALU.add)
            pE = pst(128, 3)
            for h in range(2):
                pr = slice(64 * h, 64 * (h + 1))
                nc.tensor.matmul(pE[pr, 0:3], zsb[pr, :], rs_bf[pr, c, :],
                                 tile_position=(64 * h, 64 * h))
            # bf += rs*rowsum - Z@rs
            tmp = smallp.tile([128, 3], F32, tag="tmpE")
            nc.vector.tensor_scalar(tmp[:], rs_sb[:, c, :], s2[:, 0:1],
                                    None, ALU.mult)
            nc.vector.tensor_sub(tmp[:], tmp[:], pE[:, 0:3])
            nc.vector.tensor_add(bf[:, c, :], bf[:, c, :], tmp[:])

        # ---- final combine + store ----
        for c in range(NCH):
            o = smallp.tile([128, 3], F32, tag="oc")
            nc.vector.tensor_scalar(o[:], bf[:, c, :], cut[:, c:c + 1],
                                    1e-4, ALU.mult, ALU.mult)
            nc.vector.tensor_add(o[:], o[:], rs_sb[:, c, :])
            nc.sync.dma_start(out_d[128 * c:128 * (c + 1), :], o[:])


def prep_inputs(rs, xs, coords, We1, be1, We2, be2, We3, be3,
                Wn1, bn1, Wn2, bn2, Wn3, bn3):
    """Host-side: shard rs/xs over cores, fold -ln2 into biases, pack."""
    import ml_dtypes

    f = np.float32
    bf = ml_dtypes.bfloat16
    rs = np.asarray(rs, f)
    xs = np.asarray(xs, f)
    coords = np.asarray(coords, f)
    be2a = (np.asarray(be2, f) - LN2 * np.asarray(We2, f).sum(0)).reshape(5, 1)
    be3a = (np.asarray(be3, f) - LN2 * np.asarray(We3, f).sum(0)).reshape(1, 1)
    bn2a = (np.asarray(bn2, f) - LN2 * np.asarray(Wn2, f).sum(0)).reshape(20, 1)
    bn3a = (np.asarray(bn3, f) - LN2 * np.asarray(Wn3, f).sum(0)).reshape(8, 1)
    oc = np.concatenate([np.ones((8, 1), f), coords], axis=1)
    coordsB = np.tile(coords.reshape(1, 24), (128, 1)).astype(f)
    eye = np.eye(128, dtype=bf)
    shared = dict(
        We1=np.ascontiguousarray(np.asarray(We1, f), bf),
        be1=np.asarray(be1, f).reshape(25, 1),
        We2=np.ascontiguousarray(np.asarray(We2, f), bf), be2a=be2a,
        We3=np.ascontiguousarray(np.asarray(We3, f), bf), be3a=be3a,
        Wn1=np.ascontiguousarray(np.asarray(Wn1, f), bf),
        bn1=np.asarray(bn1, f).reshape(51, 1),
        Wn2=np.ascontiguousarray(np.asarray(Wn2, f), bf), bn2a=bn2a,
        Wn3=np.ascontiguousarray(np.asarray(Wn3, f), bf), bn3a=bn3a,
        OC=np.ascontiguousarray(oc.astype(bf)), coordsB=coordsB,
        eye=eye, eye4=np.eye(4, dtype=f),
    )
    in_maps = []
    for i in range(N_CORES):
        m = dict(shared)
        m["rs"] = np.ascontiguousarray(rs[BS * i:BS * (i + 1)].reshape(R, 3))
        m["xs"] = np.ascontiguousarray(xs[BS * i:BS * (i + 1)].reshape(R, D))
        in_maps.append(m)
    return in_maps


def get_graph():
    if "nc" not in _CACHE:
        _CACHE["nc"] = build_graph()
    return _CACHE["nc"]


def kernel(**inputs):
    from concourse.bass_utils import run_bass_kernel_spmd

    nc = get_graph()
    in_maps = prep_inputs(**inputs)
    res = run_bass_kernel_spmd(nc, in_maps, core_ids=list(range(N_CORES)))
    outs = [res.results[i]["out"].reshape(BS, N, 3) for i in range(N_CORES)]
    return np.concatenate(outs, axis=0)
